# revision 5
# baseline (speedup 1.0000x reference)
"""Multi-head attention (b=4, n=4096, d_model=768, 16 heads x 128) on 8 TRN2
NeuronCores — v2: head-pipelined, fp8-DoubleRow PV.

Sharding: core c handles batch c//2, head-group c%2 (8 heads). Host sends
xT = x.T per batch (bf16), per-group weight slices, biases; host sums the two
head-group partial outputs per batch and adds the output bias.

Per-core dataflow (single pipelined instruction stream, no DRAM scratch):
  For each head h (8 per core), pipelined one head ahead:
    proj(h):  kh[d,n] / qh[d,qt] via W-stationary matmuls (bf16, N=512);
              vh[kin, kb, d] via xT-stationary matmuls (bf16, N=128), output
              scaled x16 to e4m3.
    attn(h):  per 512-wide q-tile: scores = kh_blk.T @ qh (bf16, N=512) ->
              ScalarE exp(s*SCALE + ln8) -> probs e4m3 [kin, kb, q];
              PV = sum_kb vh_pair.T @ probs_pair as fp8 DoubleRow matmuls
              (0.5 cyc/row) accumulating attnT [d, q] in PSUM;
              denominator: DVE fp16 chain over kb + f32 folds -> ones-matmul
              (f32r, 1 cyc/row) -> reciprocal -> normalize to at[h] (bf16).
    oproj:    interleaved into head 7's q-tile loop: out[nb] = sum_h
              at[h].T @ wo[h] (bf16), fp32 out, DMA per 128-row block.

ScalarE exp (~1.03 ms) paces the pipeline; projection/O-proj TensorE work
hides in its shadow. Probs x8 stays under TRN e4m3's 240 max; v x16 in e4m3;
the 1/(8*16) unscale folds into the ones value (16.0).
"""

import numpy as np
import ml_dtypes
from contextlib import ExitStack

import concourse.bass as bass
import concourse.mybir as mybir
import concourse.tile as tile
from concourse import bacc
from concourse.bass_utils import run_bass_kernel_spmd

B = 4
N_CTX = 4096
DM = 768
CH = 1024          # channels per core (8 heads x 128)
HD = 128
NH = 8             # heads per core
KC = DM // 128     # 6 contraction chunks for projections
NT = 512           # n-tile width
SCALE = HD ** -0.5
PSC = 8.0          # prob scale into e4m3
VSC = 16.0         # v scale into e4m3
LN_PSC = float(np.log(PSC))

f32 = mybir.dt.float32
f32r = mybir.dt.float32r
f16 = mybir.dt.float16
bf16 = mybir.dt.bfloat16
e4m3 = mybir.dt.float8e4
AF = mybir.ActivationFunctionType
ALU = mybir.AluOpType
DR = mybir.MatmulPerfMode.DoubleRow

_CACHE = {}


def _install_profhook():
    import contextlib, ctypes, sys, types

    if "antenv.axon_hooks" in sys.modules:
        return
    so = "/opt/axon/libaxon_pjrt.so"
    try:
        lib = ctypes.CDLL(so)
    except OSError:
        return
    if not hasattr(lib, "axon_start_nrt_profile"):
        return
    lib.axon_start_nrt_profile.argtypes = [ctypes.POINTER(ctypes.c_int64), ctypes.c_size_t]
    lib.axon_start_nrt_profile.restype = ctypes.c_int64
    lib.axon_stop_nrt_profile.argtypes = [ctypes.c_char_p]
    lib.axon_stop_nrt_profile.restype = ctypes.c_int64

    @contextlib.contextmanager
    def _hook(output_dir, device_ids):
        import jax
        jax.devices()
        if device_ids:
            ids = (ctypes.c_int64 * len(device_ids))(*device_ids)
            rc = lib.axon_start_nrt_profile(ids, len(device_ids))
        else:
            rc = lib.axon_start_nrt_profile(None, 0)
        if rc != 0:
            raise RuntimeError(f"axon_start_nrt_profile rc={rc}")
        try:
            yield
        finally:
            nf = lib.axon_stop_nrt_profile(str(output_dir).encode())
            print(f"profile: {nf} ntff file(s) in {output_dir}", file=sys.stderr)

    mod = types.ModuleType("antenv.axon_hooks")
    mod.get_axon_ntff_profile_hook = lambda: _hook
    mod.set_axon_ntff_profile_hook = lambda h: None
    sys.modules["antenv.axon_hooks"] = mod

    import concourse.bass_utils as bu
    bu.upload_artifacts = lambda tmpdir: "local://" + str(tmpdir)


def _build(n=N_CTX):
    nqt = n // NT          # q tiles
    nkb = n // 128         # key blocks
    nc = bacc.Bacc(None, target_bir_lowering=False, debug=False, num_devices=8)

    qT = nc.declare_dram_parameter("qT", [DM, n], bf16, isOutput=False)
    kT = nc.declare_dram_parameter("kT", [DM, n], bf16, isOutput=False)
    vT = nc.declare_dram_parameter("vT", [DM, n], bf16, isOutput=False)
    wq = nc.declare_dram_parameter("wq", [DM, CH], bf16, isOutput=False)
    wk = nc.declare_dram_parameter("wk", [DM, CH], bf16, isOutput=False)
    wv = nc.declare_dram_parameter("wv", [DM, CH], bf16, isOutput=False)
    wo = nc.declare_dram_parameter("wo", [CH, DM], bf16, isOutput=False)
    bqk = nc.declare_dram_parameter("bqk", [128, 2 * NH], f32, isOutput=False)
    bvr = nc.declare_dram_parameter("bvr", [128, CH], bf16, isOutput=False)
    out = nc.declare_dram_parameter("out", [n, DM], f32, isOutput=True)

    with tile.TileContext(nc) as tc, ExitStack() as ctx:
        singles = ctx.enter_context(tc.tile_pool(name="singles", bufs=1))

        wq_t = singles.tile([128, KC, CH], bf16, tag="wq")
        wk_t = singles.tile([128, KC, CH], bf16, tag="wk")
        wv_t = singles.tile([128, KC, CH], bf16, tag="wv")
        for w_t, w in ((wq_t, wq), (wk_t, wk), (wv_t, wv)):
            nc.sync.dma_start(
                out=w_t[:, :, :], in_=w[:].rearrange("(c p) m -> p c m", p=128)
            )
        wo_t = singles.tile([128, NH, DM], bf16, tag="wo")
        nc.sync.dma_start(
            out=wo_t[:, :, :], in_=wo[:].rearrange("(c p) m -> p c m", p=128)
        )
        bqk_t = singles.tile([128, 2 * NH], f32, tag="bqk")
        nc.sync.dma_start(out=bqk_t[:, :], in_=bqk[:, :])
        bvr_t = singles.tile([128, CH], bf16, tag="bvr")
        nc.sync.dma_start(out=bvr_t[:, :], in_=bvr[:, :])
        ones_f = singles.tile([128, 128], f32, tag="onesf")
        nc.vector.memset(ones_f[:, :], VSC)
        ones_t = singles.tile([128, 128], f32r, tag="ones")
        nc.vector.tensor_copy(ones_t[:, :], ones_f[:, :])
        lnp_t = singles.tile([128, 1], f32, tag="lnp")
        nc.vector.memset(lnp_t[:, :], LN_PSC)

        # pools
        xs = ctx.enter_context(tc.tile_pool(name="xs", bufs=2))
        qhp = ctx.enter_context(tc.tile_pool(name="qhp", bufs=2))
        khp = ctx.enter_context(tc.tile_pool(name="khp", bufs=2))
        vhp = ctx.enter_context(tc.tile_pool(name="vhp", bufs=2))
        pqp = ctx.enter_context(tc.tile_pool(name="pqp", bufs=4))
        dnp = ctx.enter_context(tc.tile_pool(name="dnp", bufs=1))
        atp = ctx.enter_context(tc.tile_pool(name="atp", bufs=NH))
        ostp = ctx.enter_context(tc.tile_pool(name="ostp", bufs=1))
        # PSUM: scp 2x6KB (banks 0-5), pvp 2KB (bank 6), aux 2KB (bank 7)
        scp = ctx.enter_context(tc.tile_pool(name="scp", bufs=2, space="PSUM"))
        pvp = ctx.enter_context(tc.tile_pool(name="pvp", bufs=1, space="PSUM"))
        aux = ctx.enter_context(tc.tile_pool(name="aux", bufs=1, space="PSUM"))

        xsrc = {"q": qT, "k": kT, "v": vT}

        def xtile(which, qt):
            x_t = xs.tile([128, KC, NT], bf16, tag="xs", name=f"x{which}{qt}")
            nc.sync.dma_start(
                out=x_t[:, :, :],
                in_=xsrc[which][:].rearrange("(c p) m -> p c m", p=128)[
                    :, :, qt * NT : (qt + 1) * NT
                ],
            )
            return x_t

        def kproj_chunk(hn, qt, kh_t):
            x_t = xtile("k", qt)
            ps = aux.tile([128, NT], f32, tag="aux", name="kps")
            for c in range(KC):
                nc.tensor.matmul(
                    ps[:, :],
                    wk_t[:, c, hn * HD : (hn + 1) * HD],
                    x_t[:, c, :],
                    start=(c == 0),
                    stop=(c == KC - 1),
                )
            nc.vector.tensor_scalar_add(
                kh_t[:, qt * NT : (qt + 1) * NT], ps[:, :],
                bqk_t[:, NH + hn : NH + hn + 1],
            )

        def vproj_chunk(hn, qt, vh_t):
            x_t = xtile("v", qt)
            for j in range(4):
                nb = qt * 4 + j
                ps = aux.tile([128, HD], f32, tag="aux", name="vps")
                for c in range(KC):
                    nc.tensor.matmul(
                        ps[:, :],
                        x_t[:, c, j * HD : (j + 1) * HD],
                        wv_t[:, c, hn * HD : (hn + 1) * HD],
                        start=(c == 0),
                        stop=(c == KC - 1),
                    )
                nc.vector.scalar_tensor_tensor(
                    vh_t[:, nb, :], ps[:, :], VSC,
                    bvr_t[:, hn * HD : (hn + 1) * HD],
                    op0=ALU.mult, op1=ALU.add,
                )

        def qproj(hn, qt):
            x_t = xtile("q", qt)
            ps = aux.tile([128, NT], f32, tag="aux", name="qps")
            for c in range(KC):
                nc.tensor.matmul(
                    ps[:, :],
                    wq_t[:, c, hn * HD : (hn + 1) * HD],
                    x_t[:, c, :],
                    start=(c == 0),
                    stop=(c == KC - 1),
                )
            qh_t = qhp.tile([128, NT], bf16, tag="qh", name="qh")
            nc.vector.tensor_scalar_add(
                qh_t[:, :], ps[:, :], bqk_t[:, hn : hn + 1]
            )
            return qh_t

        def scores_exp(qt, qh_t, kh_t):
            halves = []
            for half in range(2):
                pqh = pqp.tile([128, nkb // 2, NT], e4m3, tag="pq", name="pq")
                halves.append(pqh)
                off = 0
                while off < nkb // 2:
                    gsz = min(3, nkb // 2 - off)
                    sc = scp.tile([128, 3, NT], f32, tag="sc", name="sc")
                    for i in range(gsz):
                        kb = half * (nkb // 2) + off + i
                        nc.tensor.matmul(
                            sc[:, i, :],
                            kh_t[:, kb * 128 : (kb + 1) * 128],
                            qh_t[:, :],
                            start=True,
                            stop=True,
                        )
                    nc.scalar.activation(
                        pqh[:, off : off + gsz, :], sc[:, 0:gsz, :],
                        AF.Exp, bias=lnp_t[:, :], scale=SCALE,
                    )
                    off += gsz
            return halves

        def pv_denom_norm(qt, halves, vh_t, at_t):
            pv = pvp.tile([128, NT], f32, tag="pv", name="pv")
            ng = nkb // 2  # pair-matmuls
            for g in range(ng):
                pqh = halves[g // (ng // 2)]
                gg = g % (ng // 2)
                nc.tensor.matmul(
                    pv[:, :],
                    vh_t[:, 2 * g : 2 * g + 2, :],
                    pqh[:, 2 * gg : 2 * gg + 2, :],
                    start=(g == 0),
                    stop=(g == ng - 1),
                    perf_mode=DR,
                )
            # denominator: fp16 chain over kb groups of 4, alternating buffers
            tags = ("da", "db")
            groups = [
                (h, lo) for h in halves for lo in range(0, nkb // 2, 4)
            ]
            r = dnp.tile([128, 4, NT], f16, tag=tags[0], name="dch")
            nc.vector.tensor_add(
                r[:, :, :],
                groups[0][0][:, groups[0][1] : groups[0][1] + 4, :],
                groups[1][0][:, groups[1][1] : groups[1][1] + 4, :],
            )
            for i, (src, lo) in enumerate(groups[2:], start=1):
                r2 = dnp.tile([128, 4, NT], f16, tag=tags[i % 2], name="dch")
                nc.vector.tensor_add(r2[:, :, :], r[:, :, :], src[:, lo : lo + 4, :])
                r = r2
            u1 = dnp.tile([128, NT], f32, tag="t1a", name="u1")
            nc.vector.tensor_add(u1[:, :], r[:, 0, :], r[:, 1, :])
            u2 = dnp.tile([128, NT], f32, tag="t1b", name="u2")
            nc.vector.tensor_add(u2[:, :], r[:, 2, :], r[:, 3, :])
            t1 = dnp.tile([128, NT], f32r, tag="t1c", name="t1")
            nc.vector.tensor_add(t1[:, :], u1[:, :], u2[:, :])
            dn = aux.tile([128, NT], f32, tag="aux", name="dn")
            nc.tensor.matmul(
                dn[:, :], ones_t[:, :], t1[:, :],
                start=True, stop=True,
            )
            rc = dnp.tile([128, NT], f32, tag="rc", name="rc")
            nc.vector.reciprocal(rc[:, :], dn[:, :])
            nc.vector.tensor_mul(
                at_t[:, qt * NT : (qt + 1) * NT], pv[:, :], rc[:, :]
            )

        def oproj(nb, at_ts):
            ost = ostp.tile([128, DM], f32, tag="ost", name="ost")
            po = aux.tile([128, NT], f32, tag="aux", name="po")
            for hn in range(NH):
                nc.tensor.matmul(
                    po[:, :], at_ts[hn][:, nb * 128 : (nb + 1) * 128],
                    wo_t[:, hn, 0:NT],
                    start=(hn == 0), stop=(hn == NH - 1), skip_group_check=True,
                )
            nc.vector.tensor_copy(ost[:, 0:NT], po[:, :])
            po2 = aux.tile([128, DM - NT], f32, tag="aux", name="po2")
            for hn in range(NH):
                nc.tensor.matmul(
                    po2[:, :], at_ts[hn][:, nb * 128 : (nb + 1) * 128],
                    wo_t[:, hn, NT:DM],
                    start=(hn == 0), stop=(hn == NH - 1), skip_group_check=True,
                )
            nc.vector.tensor_copy(ost[:, NT:DM], po2[:, :])
            nc.sync.dma_start(out=out[nb * 128 : (nb + 1) * 128, :], in_=ost[:, :])

        # ---------------- pipelined main program ----------------
        kh_t = khp.tile([128, n], bf16, tag="kh", name="kh")
        vh_t = vhp.tile([128, nkb, HD], e4m3, tag="vh", name="vh")
        for qt in range(nqt):
            kproj_chunk(0, qt, kh_t)
            vproj_chunk(0, qt, vh_t)
        qh_next = qproj(0, 0)

        at_ts = []
        for hn in range(NH):
            kh_cur, vh_cur = kh_t, vh_t
            if hn < NH - 1:
                kh_t = khp.tile([128, n], bf16, tag="kh", name="kh")
                vh_t = vhp.tile([128, nkb, HD], e4m3, tag="vh", name="vh")
            at_t = atp.tile([128, n], bf16, tag="at", name="at")
            at_ts.append(at_t)
            prev_halves = None
            for qt in range(nqt + 1):
                if qt < nqt:
                    if hn < NH - 1:
                        kproj_chunk(hn + 1, qt, kh_t)
                        vproj_chunk(hn + 1, qt, vh_t)
                    qh_cur = qh_next
                    if qt < nqt - 1:
                        qh_next = qproj(hn, qt + 1)
                    elif hn < NH - 1:
                        qh_next = qproj(hn + 1, 0)
                    halves = scores_exp(qt, qh_cur, kh_cur)
                if qt > 0:
                    pv_denom_norm(qt - 1, prev_halves, vh_cur, at_t)
                    if hn == NH - 1:
                        for j in range(4):
                            oproj((qt - 1) * 4 + j, at_ts)
                if qt < nqt:
                    prev_halves = halves

    nc.compile()
    return nc


def _get_nc(n=N_CTX):
    if n not in _CACHE:
        _CACHE[n] = _build(n)
    return _CACHE[n]


def _shard_inputs(q, k, v, Wq, bq, Wk, bk, Wv, bv, Wo, bo):
    bf = ml_dtypes.bfloat16
    in_maps = []
    for c in range(8):
        bi, hg = c // 2, c % 2
        s = slice(hg * CH, (hg + 1) * CH)
        bqk_c = np.empty((128, 2 * NH), np.float32)
        for h in range(NH):
            bqk_c[:, h] = bq[hg * CH + h * HD : hg * CH + (h + 1) * HD]
            bqk_c[:, NH + h] = bk[hg * CH + h * HD : hg * CH + (h + 1) * HD]
        in_maps.append({
            "qT": np.ascontiguousarray(q[bi].T).astype(bf),
            "kT": np.ascontiguousarray(k[bi].T).astype(bf),
            "vT": np.ascontiguousarray(v[bi].T).astype(bf),
            "wq": np.ascontiguousarray(Wq[:, s]).astype(bf),
            "wk": np.ascontiguousarray(Wk[:, s]).astype(bf),
            "wv": np.ascontiguousarray(Wv[:, s]).astype(bf),
            "wo": np.ascontiguousarray(Wo[s, :]).astype(bf),
            "bqk": bqk_c,
            "bvr": np.ascontiguousarray(
                np.broadcast_to((VSC * bv[s]).astype(np.float32), (128, CH))
            ).astype(bf),
        })
    return in_maps


def kernel(q, k, v, Wq, bq, Wk, bk, Wv, bv, Wo, bo, _profile=False):
    import os

    q = np.asarray(q); k = np.asarray(k); v = np.asarray(v)
    n = q.shape[1]
    nc = _get_nc(n)
    in_maps = _shard_inputs(
        q, k, v, np.asarray(Wq), np.asarray(bq), np.asarray(Wk), np.asarray(bk),
        np.asarray(Wv), np.asarray(bv), np.asarray(Wo), np.asarray(bo),
    )
    profile = _profile or bool(int(os.environ.get("KERNEL_PROFILE", "0")))
    if profile:
        _install_profhook()
    res = run_bass_kernel_spmd(nc, in_maps, list(range(8)), trace=profile)
    if profile and res.exec_time_ns is not None:
        print(f"HW exec time: {res.exec_time_ns} ns")
    bo32 = np.asarray(bo, np.float32)
    out = np.empty((q.shape[0], n, DM), np.float32)
    for bi in range(q.shape[0]):
        out[bi] = res.results[2 * bi]["out"] + res.results[2 * bi + 1]["out"] + bo32
    return out


# revision 6
# speedup vs baseline: 1.2594x; 1.2594x over previous
"""Multi-head attention (b=4, n=4096, d_model=768, 16 heads x 128) on 8 TRN2
NeuronCores — v2: head-pipelined, fp8-DoubleRow PV.

Sharding: core c handles batch c//2, head-group c%2 (8 heads). Host sends
xT = x.T per batch (bf16), per-group weight slices, biases; host sums the two
head-group partial outputs per batch and adds the output bias.

Per-core dataflow (single pipelined instruction stream, no DRAM scratch):
  For each head h (8 per core), pipelined one head ahead:
    proj(h):  kh[d,n] / qh[d,qt] via W-stationary matmuls (bf16, N=512);
              vh[kin, kb, d] via xT-stationary matmuls (bf16, N=128), output
              scaled x16 to e4m3.
    attn(h):  per 512-wide q-tile: scores = kh_blk.T @ qh (bf16, N=512) ->
              ScalarE exp(s*SCALE + ln8) -> probs e4m3 [kin, kb, q];
              PV = sum_kb vh_pair.T @ probs_pair as fp8 DoubleRow matmuls
              (0.5 cyc/row) accumulating attnT [d, q] in PSUM;
              denominator: DVE fp16 chain over kb + f32 folds -> ones-matmul
              (f32r, 1 cyc/row) -> reciprocal -> normalize to at[h] (bf16).
    oproj:    interleaved into head 7's q-tile loop: out[nb] = sum_h
              at[h].T @ wo[h] (bf16), fp32 out, DMA per 128-row block.

ScalarE exp (~1.03 ms) paces the pipeline; projection/O-proj TensorE work
hides in its shadow. Probs x8 stays under TRN e4m3's 240 max; v x16 in e4m3;
the 1/(8*16) unscale folds into the ones value (16.0).
"""

import numpy as np
import ml_dtypes
from contextlib import ExitStack

import concourse.bass as bass
import concourse.mybir as mybir
import concourse.tile as tile
from concourse import bacc
from concourse.bass_utils import run_bass_kernel_spmd

B = 4
N_CTX = 4096
DM = 768
CH = 1024          # channels per core (8 heads x 128)
HD = 128
NH = 8             # heads per core
KC = DM // 128     # 6 contraction chunks for projections
NT = 512           # n-tile width
SCALE = HD ** -0.5
PSC = 8.0          # prob scale into e4m3
VSC = 16.0         # v scale into e4m3
LN_PSC = float(np.log(PSC))

f32 = mybir.dt.float32
f32r = mybir.dt.float32r
f16 = mybir.dt.float16
bf16 = mybir.dt.bfloat16
e4m3 = mybir.dt.float8e4
AF = mybir.ActivationFunctionType
ALU = mybir.AluOpType
DR = mybir.MatmulPerfMode.DoubleRow

_CACHE = {}


def _install_profhook():
    import contextlib, ctypes, sys, types

    if "antenv.axon_hooks" in sys.modules:
        return
    so = "/opt/axon/libaxon_pjrt.so"
    try:
        lib = ctypes.CDLL(so)
    except OSError:
        return
    if not hasattr(lib, "axon_start_nrt_profile"):
        return
    lib.axon_start_nrt_profile.argtypes = [ctypes.POINTER(ctypes.c_int64), ctypes.c_size_t]
    lib.axon_start_nrt_profile.restype = ctypes.c_int64
    lib.axon_stop_nrt_profile.argtypes = [ctypes.c_char_p]
    lib.axon_stop_nrt_profile.restype = ctypes.c_int64

    @contextlib.contextmanager
    def _hook(output_dir, device_ids):
        import jax
        jax.devices()
        if device_ids:
            ids = (ctypes.c_int64 * len(device_ids))(*device_ids)
            rc = lib.axon_start_nrt_profile(ids, len(device_ids))
        else:
            rc = lib.axon_start_nrt_profile(None, 0)
        if rc != 0:
            raise RuntimeError(f"axon_start_nrt_profile rc={rc}")
        try:
            yield
        finally:
            nf = lib.axon_stop_nrt_profile(str(output_dir).encode())
            print(f"profile: {nf} ntff file(s) in {output_dir}", file=sys.stderr)

    mod = types.ModuleType("antenv.axon_hooks")
    mod.get_axon_ntff_profile_hook = lambda: _hook
    mod.set_axon_ntff_profile_hook = lambda h: None
    sys.modules["antenv.axon_hooks"] = mod

    import concourse.bass_utils as bu
    bu.upload_artifacts = lambda tmpdir: "local://" + str(tmpdir)


def _build(n=N_CTX):
    nqt = n // NT          # q tiles
    nkb = n // 128         # key blocks
    nc = bacc.Bacc(None, target_bir_lowering=False, debug=False, num_devices=8)

    qT = nc.declare_dram_parameter("qT", [DM, n], bf16, isOutput=False)
    kT = nc.declare_dram_parameter("kT", [DM, n], bf16, isOutput=False)
    vT = nc.declare_dram_parameter("vT", [DM, n], bf16, isOutput=False)
    wq = nc.declare_dram_parameter("wq", [DM, CH], bf16, isOutput=False)
    wk = nc.declare_dram_parameter("wk", [DM, CH], bf16, isOutput=False)
    wv = nc.declare_dram_parameter("wv", [DM, CH], bf16, isOutput=False)
    wo = nc.declare_dram_parameter("wo", [CH, DM], bf16, isOutput=False)
    bqk = nc.declare_dram_parameter("bqk", [128, 2 * NH], f32, isOutput=False)
    bvr = nc.declare_dram_parameter("bvr", [128, CH], bf16, isOutput=False)
    out = nc.declare_dram_parameter("out", [n, DM], f32, isOutput=True)

    with tile.TileContext(nc) as tc, ExitStack() as ctx:
        singles = ctx.enter_context(tc.tile_pool(name="singles", bufs=1))

        wq_t = singles.tile([128, KC, CH], bf16, tag="wq")
        wk_t = singles.tile([128, KC, CH], bf16, tag="wk")
        wv_t = singles.tile([128, KC, CH], bf16, tag="wv")
        for w_t, w in ((wq_t, wq), (wk_t, wk), (wv_t, wv)):
            nc.sync.dma_start(
                out=w_t[:, :, :], in_=w[:].rearrange("(c p) m -> p c m", p=128)
            )
        wo_t = singles.tile([128, NH, DM], bf16, tag="wo")
        nc.sync.dma_start(
            out=wo_t[:, :, :], in_=wo[:].rearrange("(c p) m -> p c m", p=128)
        )
        bqk_t = singles.tile([128, 2 * NH], f32, tag="bqk")
        nc.sync.dma_start(out=bqk_t[:, :], in_=bqk[:, :])
        bvr_t = singles.tile([128, CH], bf16, tag="bvr")
        nc.sync.dma_start(out=bvr_t[:, :], in_=bvr[:, :])
        ones8_t = singles.tile([128, 2, 128], e4m3, tag="ones8")
        nc.vector.memset(ones8_t[:, :, :], VSC)
        lnp_t = singles.tile([128, 1], f32, tag="lnp")
        nc.vector.memset(lnp_t[:, :], LN_PSC)

        # pools
        xs = ctx.enter_context(tc.tile_pool(name="xs", bufs=3))
        qhp = ctx.enter_context(tc.tile_pool(name="qhp", bufs=2))
        khp = ctx.enter_context(tc.tile_pool(name="khp", bufs=2))
        vhp = ctx.enter_context(tc.tile_pool(name="vhp", bufs=2))
        pqp = ctx.enter_context(tc.tile_pool(name="pqp", bufs=4))
        dnp = ctx.enter_context(tc.tile_pool(name="dnp", bufs=1))
        atp = ctx.enter_context(tc.tile_pool(name="atp", bufs=NH))
        ostp = ctx.enter_context(tc.tile_pool(name="ostp", bufs=1))
        # PSUM: scp 2x6KB (banks 0-5), pvp 2KB (bank 6), aux 2KB (bank 7)
        scp = ctx.enter_context(tc.tile_pool(name="scp", bufs=2, space="PSUM"))
        pvp = ctx.enter_context(tc.tile_pool(name="pvp", bufs=1, space="PSUM"))
        aux = ctx.enter_context(tc.tile_pool(name="aux", bufs=1, space="PSUM"))

        xsrc = {"q": qT, "k": kT, "v": vT}

        def xtile(which, qt):
            x_t = xs.tile([128, KC, NT], bf16, tag="xs", name=f"x{which}{qt}")
            nc.sync.dma_start(
                out=x_t[:, :, :],
                in_=xsrc[which][:].rearrange("(c p) m -> p c m", p=128)[
                    :, :, qt * NT : (qt + 1) * NT
                ],
            )
            return x_t

        def kproj_chunk(hn, qt, kh_t):
            x_t = xtile("k", qt)
            ps = aux.tile([128, NT], f32, tag="aux", name="kps")
            for c in range(KC):
                nc.tensor.matmul(
                    ps[:, :],
                    wk_t[:, c, hn * HD : (hn + 1) * HD],
                    x_t[:, c, :],
                    start=(c == 0),
                    stop=(c == KC - 1),
                )
            nc.vector.tensor_scalar_add(
                kh_t[:, qt * NT : (qt + 1) * NT], ps[:, :],
                bqk_t[:, NH + hn : NH + hn + 1],
            )

        def vproj_chunk(hn, qt, vh_t):
            x_t = xtile("v", qt)
            for j in range(4):
                nb = qt * 4 + j
                ps = aux.tile([128, HD], f32, tag="aux", name="vps")
                for c in range(KC):
                    nc.tensor.matmul(
                        ps[:, :],
                        x_t[:, c, j * HD : (j + 1) * HD],
                        wv_t[:, c, hn * HD : (hn + 1) * HD],
                        start=(c == 0),
                        stop=(c == KC - 1),
                    )
                nc.vector.scalar_tensor_tensor(
                    vh_t[:, nb, :], ps[:, :], VSC,
                    bvr_t[:, hn * HD : (hn + 1) * HD],
                    op0=ALU.mult, op1=ALU.add,
                )

        def qproj(hn, qt):
            x_t = xtile("q", qt)
            ps = aux.tile([128, NT], f32, tag="aux", name="qps")
            for c in range(KC):
                nc.tensor.matmul(
                    ps[:, :],
                    wq_t[:, c, hn * HD : (hn + 1) * HD],
                    x_t[:, c, :],
                    start=(c == 0),
                    stop=(c == KC - 1),
                )
            qh_t = qhp.tile([128, NT], bf16, tag="qh", name="qh")
            nc.vector.tensor_scalar_add(
                qh_t[:, :], ps[:, :], bqk_t[:, hn : hn + 1]
            )
            return qh_t

        def scores_exp(qt, qh_t, kh_t):
            halves = []
            for half in range(2):
                pqh = pqp.tile([128, nkb // 2, NT], e4m3, tag="pq", name="pq")
                halves.append(pqh)
                off = 0
                while off < nkb // 2:
                    gsz = min(3, nkb // 2 - off)
                    sc = scp.tile([128, 3, NT], f32, tag="sc", name="sc")
                    for i in range(gsz):
                        kb = half * (nkb // 2) + off + i
                        nc.tensor.matmul(
                            sc[:, i, :],
                            kh_t[:, kb * 128 : (kb + 1) * 128],
                            qh_t[:, :],
                            start=True,
                            stop=True,
                        )
                    nc.scalar.activation(
                        pqh[:, off : off + gsz, :], sc[:, 0:gsz, :],
                        AF.Exp, bias=lnp_t[:, :], scale=SCALE,
                    )
                    off += gsz
            return halves

        def pv_denom_norm(qt, halves, vh_t, at_t):
            pv = pvp.tile([128, NT], f32, tag="pv", name="pv")
            ng = nkb // 2  # pair-matmuls
            for g in range(ng):
                pqh = halves[g // (ng // 2)]
                gg = g % (ng // 2)
                nc.tensor.matmul(
                    pv[:, :],
                    vh_t[:, 2 * g : 2 * g + 2, :],
                    pqh[:, 2 * gg : 2 * gg + 2, :],
                    start=(g == 0),
                    stop=(g == ng - 1),
                    perf_mode=DR,
                )
            # denominator: fp8 DoubleRow ones-matmuls (stationary never changes)
            dn = aux.tile([128, NT], f32, tag="aux", name="dn")
            for g in range(ng):
                pqh = halves[g // (ng // 2)]
                gg = g % (ng // 2)
                nc.tensor.matmul(
                    dn[:, :],
                    ones8_t[:, :, :],
                    pqh[:, 2 * gg : 2 * gg + 2, :],
                    start=(g == 0),
                    stop=(g == ng - 1),
                    perf_mode=DR,
                )
            rc = dnp.tile([128, NT], f32, tag="rc", name="rc")
            nc.vector.reciprocal_approx_fast(rc[:, :], dn[:, :])
            nc.vector.tensor_mul(
                at_t[:, qt * NT : (qt + 1) * NT], pv[:, :], rc[:, :]
            )

        def oproj(nb, at_ts):
            ost = ostp.tile([128, DM], f32, tag="ost", name="ost")
            po = aux.tile([128, NT], f32, tag="aux", name="po")
            for hn in range(NH):
                nc.tensor.matmul(
                    po[:, :], at_ts[hn][:, nb * 128 : (nb + 1) * 128],
                    wo_t[:, hn, 0:NT],
                    start=(hn == 0), stop=(hn == NH - 1), skip_group_check=True,
                )
            nc.vector.tensor_copy(ost[:, 0:NT], po[:, :])
            po2 = aux.tile([128, DM - NT], f32, tag="aux", name="po2")
            for hn in range(NH):
                nc.tensor.matmul(
                    po2[:, :], at_ts[hn][:, nb * 128 : (nb + 1) * 128],
                    wo_t[:, hn, NT:DM],
                    start=(hn == 0), stop=(hn == NH - 1), skip_group_check=True,
                )
            nc.vector.tensor_copy(ost[:, NT:DM], po2[:, :])
            nc.sync.dma_start(out=out[nb * 128 : (nb + 1) * 128, :], in_=ost[:, :])

        # ---------------- pipelined main program ----------------
        kh_t = khp.tile([128, n], bf16, tag="kh", name="kh")
        vh_t = vhp.tile([128, nkb, HD], e4m3, tag="vh", name="vh")
        for qt in range(nqt):
            kproj_chunk(0, qt, kh_t)
            vproj_chunk(0, qt, vh_t)
        qh_next = qproj(0, 0)

        at_ts = []
        for hn in range(NH):
            kh_cur, vh_cur = kh_t, vh_t
            if hn < NH - 1:
                kh_t = khp.tile([128, n], bf16, tag="kh", name="kh")
                vh_t = vhp.tile([128, nkb, HD], e4m3, tag="vh", name="vh")
            at_t = atp.tile([128, n], bf16, tag="at", name="at")
            at_ts.append(at_t)
            prev_halves = None
            for qt in range(nqt + 1):
                if qt < nqt:
                    if hn < NH - 1:
                        kproj_chunk(hn + 1, qt, kh_t)
                        vproj_chunk(hn + 1, qt, vh_t)
                    qh_cur = qh_next
                    if qt < nqt - 1:
                        qh_next = qproj(hn, qt + 1)
                    elif hn < NH - 1:
                        qh_next = qproj(hn + 1, 0)
                    halves = scores_exp(qt, qh_cur, kh_cur)
                if qt > 0:
                    pv_denom_norm(qt - 1, prev_halves, vh_cur, at_t)
                    if hn == NH - 1:
                        for j in range(4):
                            oproj((qt - 1) * 4 + j, at_ts)
                if qt < nqt:
                    prev_halves = halves

    nc.compile()
    return nc


def _get_nc(n=N_CTX):
    if n not in _CACHE:
        _CACHE[n] = _build(n)
    return _CACHE[n]


def _shard_inputs(q, k, v, Wq, bq, Wk, bk, Wv, bv, Wo, bo):
    bf = ml_dtypes.bfloat16
    in_maps = []
    for c in range(8):
        bi, hg = c // 2, c % 2
        s = slice(hg * CH, (hg + 1) * CH)
        bqk_c = np.empty((128, 2 * NH), np.float32)
        for h in range(NH):
            bqk_c[:, h] = bq[hg * CH + h * HD : hg * CH + (h + 1) * HD]
            bqk_c[:, NH + h] = bk[hg * CH + h * HD : hg * CH + (h + 1) * HD]
        in_maps.append({
            "qT": np.ascontiguousarray(q[bi].T).astype(bf),
            "kT": np.ascontiguousarray(k[bi].T).astype(bf),
            "vT": np.ascontiguousarray(v[bi].T).astype(bf),
            "wq": np.ascontiguousarray(Wq[:, s]).astype(bf),
            "wk": np.ascontiguousarray(Wk[:, s]).astype(bf),
            "wv": np.ascontiguousarray(Wv[:, s]).astype(bf),
            "wo": np.ascontiguousarray(Wo[s, :]).astype(bf),
            "bqk": bqk_c,
            "bvr": np.ascontiguousarray(
                np.broadcast_to((VSC * bv[s]).astype(np.float32), (128, CH))
            ).astype(bf),
        })
    return in_maps


def kernel(q, k, v, Wq, bq, Wk, bk, Wv, bv, Wo, bo, _profile=False):
    import os

    q = np.asarray(q); k = np.asarray(k); v = np.asarray(v)
    n = q.shape[1]
    nc = _get_nc(n)
    in_maps = _shard_inputs(
        q, k, v, np.asarray(Wq), np.asarray(bq), np.asarray(Wk), np.asarray(bk),
        np.asarray(Wv), np.asarray(bv), np.asarray(Wo), np.asarray(bo),
    )
    profile = _profile or bool(int(os.environ.get("KERNEL_PROFILE", "0")))
    if profile:
        _install_profhook()
    res = run_bass_kernel_spmd(nc, in_maps, list(range(8)), trace=profile)
    if profile and res.exec_time_ns is not None:
        print(f"HW exec time: {res.exec_time_ns} ns")
    bo32 = np.asarray(bo, np.float32)
    out = np.empty((q.shape[0], n, DM), np.float32)
    for bi in range(q.shape[0]):
        out[bi] = res.results[2 * bi]["out"] + res.results[2 * bi + 1]["out"] + bo32
    return out
